# revision 27
# baseline (speedup 1.0000x reference)
"""Llama3 attention layer (T=2048, 32 q heads / 8 kv heads, D=128, hidden 4096)
on 8 Trainium2 NeuronCores, tensor-parallel over heads.

Per-core shard: 4 q heads + 1 kv head (w_qkv columns), 512 w_o rows.
Each core computes a full [T, 4096] o_proj partial in bf16; the host sums
the 8 partials in fp32 (the all-reduce of the row-parallel w_o).

Device algorithm (per core), all matmuls in bf16 with fp32 PSUM accumulation:
  1. qkv^T = w_shard^T @ hs^T          -> [768, T]  (c on partitions)
     column-block order v, k, q0..q3; w_qkv resident in SBUF (loaded once);
     hs^T streamed through a 6-deep chunk ring with one-tile lookahead
  2. RoPE on q^T/k^T rows via duplicated cos/sin tables (DVE)
  3. V = transpose(v^T) via PE transposes
  4. S^T[s,t] = k^T.T @ q^T (per head), exp on ACT; diagonal s-blocks use
     trapezoid moving slices (width 512-128r) + one [128,128] triangular mask
  5. out^T[d,t] += V[s].T @ P^T[s,t]; denom via ones-matmul;
     1/den = exp(-ln(den)) on ACT; normalize mult on DVE
  6. out[t,:] = (O^T).T @ w_o_shard    -> bf16 partial, DMA to DRAM
"""
import math

import numpy as np
import ml_dtypes

import bass_rust
import concourse.bass as bass
import concourse.mybir as mybir
import concourse.tile as tile
from concourse.bass_utils import run_bass_kernel_spmd
from concourse.masks import make_identity
from concourse.vector_clock import ScopedClock

BF16 = mybir.dt.bfloat16
F32 = mybir.dt.float32
bf16 = ml_dtypes.bfloat16

T = 2048
HID = 4096
D = 128
NQH = 4          # q heads per core
CB = 6           # qkv col blocks of 128 (v, k, 4 q heads)
HCH = HID // 128  # 32 hidden chunks
TJ = 512          # t tile width
NJ = T // TJ      # 4 t tiles
SB = T // 128     # 16 s blocks
SCALE = 1.0 / math.sqrt(D)

_MAX_CTRL_WAITS = 1


def _install_drain_fix():
    """walrus in this image allows only 1 sem wait on CTRL (nop/drain)
    instructions; spread the Tile tail-drain's global-clock waits across
    preceding sync-engine NOPs."""
    if getattr(tile.TileContext, "_drain_fix_installed", False):
        return

    def _patched(self, tick_clock, wait_clock):
        nc = self.nc
        nops = [nc.sync.nop(nofuse=True, hint=f"drainw{i}") for i in range(32)]
        drain_inst = nc.sync.drain()
        wait_clock.add_sem_waits(
            drain_inst.ins, ScopedClock({None: tick_clock.global_clock})
        )
        si = drain_inst.ins.sync_info
        waits = list(si.on_wait) if si and si.on_wait else []
        if len(waits) > _MAX_CTRL_WAITS:
            chunks = [
                waits[i:i + _MAX_CTRL_WAITS]
                for i in range(0, len(waits), _MAX_CTRL_WAITS)
            ]
            drain_inst.ins.sync_info = bass_rust.SyncInfo(
                on_wait=chunks[-1], on_update=list(si.on_update or [])
            )
            for nop, chunk in zip(nops, chunks[:-1]):
                nop.ins.sync_info = bass_rust.SyncInfo(on_wait=chunk, on_update=[])
        nc.all_engine_barrier()
        assert self.sems is not None
        popped = nc._tile_sem_poison_stack.pop()
        assert popped is self._sem_poison
        nc.clear_and_free_semaphores(list(self.sems.allocated().values()))
        nc.all_engine_barrier()

    tile.TileContext._drain_and_barrier = _patched
    tile.TileContext._drain_fix_installed = True


def _fix_bir_waits(bir_json: bytes, max_waits: int = 1) -> bytes:
    """walrus in this image accepts very few sem waits per instruction.
    Split any instruction carrying more than `max_waits` waits by inserting
    same-engine NoOps ahead of it that carry the excess waits."""
    import json

    bir = json.loads(bir_json)
    n_split = 0
    for fn in bir["functions"]:
        for blk in fn["blocks"]:
            out = []
            for inst in blk["instructions"]:
                si = inst.get("sync_info")
                waits = (si or {}).get("on_wait") or []
                if len(waits) > max_waits:
                    chunks = [
                        waits[i:i + max_waits]
                        for i in range(0, len(waits), max_waits)
                    ]
                    for k, ch in enumerate(chunks[:-1]):
                        out.append(
                            {
                                "debug": inst.get("debug", 0),
                                "engine": inst["engine"],
                                "ins": [],
                                "name": f"{inst['name']}-w{k}",
                                "opcode": "NoOp",
                                "outs": [],
                                "sync_info": {"on_update": [], "on_wait": ch},
                            }
                        )
                        n_split += 1
                    si["on_wait"] = chunks[-1]
                out.append(inst)
            blk["instructions"] = out
    return json.dumps(bir).encode()


def build_nc() -> bass.Bass:
    _install_drain_fix()
    nc = bass.Bass()

    hsT_d = nc.dram_tensor("hsT", [HID, T], BF16, kind="ExternalInput")
    wqkv_d = nc.dram_tensor("wqkv", [HID, CB * 128], BF16, kind="ExternalInput")
    wo_d = nc.dram_tensor("wo", [NQH * 128, HID], BF16, kind="ExternalInput")
    cos_d = nc.dram_tensor("cos2", [128, T], BF16, kind="ExternalInput")
    sin_d = nc.dram_tensor("sin2", [128, T], BF16, kind="ExternalInput")
    mask_d = nc.dram_tensor("masktri", [128, 128], BF16, kind="ExternalInput")
    out_d = nc.dram_tensor("out", [T, HID], BF16, kind="ExternalOutput")

    hsT_r = hsT_d.rearrange("(o p) t -> p o t", p=128)     # [128, 32, T]
    wqkv_r = wqkv_d.rearrange("(o p) c -> p o c", p=128)   # [128, 32, 768]
    wo_r = wo_d.rearrange("(o p) n -> p o n", p=128)       # [128, 4, HID]

    with tile.TileContext(nc) as tc:
        with (
            tc.tile_pool(name="const", bufs=1) as constp,
            tc.tile_pool(name="acts", bufs=1) as actp,
            tc.tile_pool(name="ps", bufs=8, space="PSUM") as psp,
            tc.tile_pool(name="hst", bufs=6) as hstp,
            tc.tile_pool(name="qs", bufs=2) as qsp,
            tc.tile_pool(name="qkt", bufs=2) as qktp,
            tc.tile_pool(name="rtmp", bufs=2) as rtp,
            tc.tile_pool(name="pp", bufs=10) as ppp,
            tc.tile_pool(name="rcp", bufs=1) as rcpp,
            tc.tile_pool(name="outp", bufs=4) as outp,
        ):
            cos_sb = constp.tile([128, T], BF16, tag="cos")
            sin_sb = constp.tile([128, T], BF16, tag="sin")
            mask_sb = constp.tile([128, 128], BF16, tag="masktri")
            ones_sb = constp.tile([128, 128], BF16, tag="ones")
            ident_sb = constp.tile([128, 128], BF16, tag="ident")
            warm_sb = constp.tile([128, TJ], BF16, tag="warm")

            # PE warmup: dummy matmuls fill the initial DMA wait so the HAM
            # clock-gate opens before real work arrives (~3.4us busy needed).
            nc.vector.memset(warm_sb[:], 0.0)
            for wi in range(14):
                pw = psp.tile([128, TJ], F32, tag="ps", name="ps_warm")
                nc.tensor.matmul(
                    pw[:], warm_sb[:, 0:128], warm_sb[:], start=True, stop=True
                )
            nc.vector.memset(ones_sb[:], 1.0)
            make_identity(nc, ident_sb[:])

            # persistent activations (per-strip tiles avoid false cross-strip
            # dependencies in the tile tracker)
            w_sb = actp.tile([128, HCH, 768], BF16, tag="w")
            wo_sb = actp.tile([128, NQH, HID], BF16, tag="wo")
            kT_sb = [
                actp.tile([128, TJ], BF16, tag=f"kT{jj}", name=f"kT{jj}")
                for jj in range(NJ)
            ]
            v_sb = [actp.tile([128, 128], BF16, tag=f"v{i}", name=f"v{i}") for i in range(SB)]

            # hs^T chunk ring: 16 chunks of [128, 8, 512], one-j lookahead
            hst_tiles = [None] * (4 * NJ)
            hst_issued = [0]

            def issue_hst_upto(limit):
                while hst_issued[0] < min(limit, 4 * NJ):
                    g = hst_issued[0]
                    tile_g = hstp.tile(
                        [128, 8, TJ], BF16, tag="hst", name=f"hst{g}"
                    )
                    if g == 0:
                        # split the very first chunk so the first QKV matmuls
                        # can start half a transfer earlier
                        nc.sync.dma_start(
                            tile_g[:, 0:4, :], hsT_r[:, 0:4, 0:TJ]
                        )
                        nc.sync.dma_start(
                            tile_g[:, 4:8, :], hsT_r[:, 4:8, 0:TJ]
                        )
                    else:
                        nc.sync.dma_start(
                            tile_g[:],
                            hsT_r[:, (g % 4) * 8:(g % 4 + 1) * 8, bass.ts(g // 4, TJ)],
                        )
                    hst_tiles[g] = tile_g
                    hst_issued[0] += 1

            ot_strip = {}  # (j, h) -> strip tile, live for one j

            def oproj_tile(t, n):
                tj, tr = t // 4, bass.ts(t % 4, 128)
                ps = psp.tile([128, TJ], F32, tag="ps", name="ps_op")
                for c in range(NQH):
                    nc.tensor.matmul(
                        ps[:], ot_strip[(tj, c)][:, tr], wo_sb[:, c, bass.ts(n, TJ)],
                        start=(c == 0), stop=(c == NQH - 1),
                    )
                ob = outp.tile([128, TJ], BF16, tag="out")
                nc.vector.tensor_copy(ob[:], ps[:])
                nc.sync.dma_start(out_d[bass.ts(t, 128), bass.ts(n, TJ)], ob[:])

            for j in range(NJ):
                js = bass.ts(j, TJ)
                # ---- QKV^T for this t tile ----
                if j == 0:
                    # interleave w octets with the first hst chunks so the
                    # first accumulation chain starts as soon as chunk 0 lands
                    nc.sync.dma_start(w_sb[:, 0:4, :], wqkv_r[:, 0:4, :])
                    issue_hst_upto(1)
                    nc.sync.dma_start(w_sb[:, 4:8, :], wqkv_r[:, 4:8, :])
                    for g in range(1, 4):
                        issue_hst_upto(g + 1)
                        nc.sync.dma_start(
                            w_sb[:, g * 8:(g + 1) * 8, :],
                            wqkv_r[:, g * 8:(g + 1) * 8, :],
                        )
                    issue_hst_upto(8)
                else:
                    issue_hst_upto(4 * (j + 1) + 2)

                # per-strip q tiles (roped q heads for this j)
                q_strip = [
                    qsp.tile([128, TJ], BF16, tag=f"q{h}", name=f"q{h}_{j}")
                    for h in range(NQH)
                ]
                vT_j = qsp.tile([128, TJ], BF16, tag="vT", name=f"vT_{j}")

                def v_transposes():
                    # PE-transpose the V strip to [s, d] blocks; deferred two
                    # chains past cb0 so the PE isn't stalled on the vT copy
                    for r in range(4):
                        pv = psp.tile([128, 128], BF16, tag="ps", name="ps_vt")
                        nc.tensor.transpose(
                            pv[:], vT_j[:, bass.ts(r, 128)], ident_sb[:]
                        )
                        nc.vector.tensor_copy(v_sb[4 * j + r][:], pv[:])

                def qkv_post(cb, ps):
                    if cb == 0:
                        # V block: copy to SBUF (transposes deferred)
                        nc.vector.tensor_copy(vT_j[:], ps[:])
                    else:
                        # k (cb=1) -> kT_sb strip; q head h (cb=2+h) -> q_strip
                        qk_t = qktp.tile([128, TJ], BF16, tag="qkt")
                        nc.vector.tensor_copy(qk_t[:], ps[:])
                        # rope: q' = q*cos2 + swap(q)*sin2 (sin2 top half negated)
                        swp = rtp.tile([128, TJ], BF16, tag="swp")
                        nc.vector.tensor_copy(swp[0:64, :], qk_t[64:128, :])
                        nc.vector.tensor_copy(swp[64:128, :], qk_t[0:64, :])
                        ta = rtp.tile([128, TJ], BF16, tag="ta")
                        nc.vector.tensor_tensor(
                            ta[:], qk_t[:], cos_sb[:, js], mybir.AluOpType.mult
                        )
                        tb = rtp.tile([128, TJ], BF16, tag="tb")
                        nc.vector.tensor_tensor(
                            tb[:], swp[:], sin_sb[:, js], mybir.AluOpType.mult
                        )
                        if cb == 1:
                            nc.vector.tensor_tensor(
                                kT_sb[j][:], ta[:], tb[:], mybir.AluOpType.add
                            )
                        else:
                            nc.vector.tensor_tensor(
                                q_strip[cb - 2][:], ta[:], tb[:],
                                mybir.AluOpType.add,
                            )

                if j == 0:
                    # DMA-bound pipeline head: octet-group outer, cb inner —
                    # 6 parallel PSUM chains give 10us of PE work per arriving
                    # (w octet, hst chunk) pair instead of 1.7us
                    ps_qkv0 = [
                        psp.tile([128, TJ], F32, tag="ps", name=f"ps_qkv0_{cb}")
                        for cb in range(CB)
                    ]
                    for g in range(4):
                        for cb in range(CB):
                            for h in range(8 * g, 8 * g + 8):
                                nc.tensor.matmul(
                                    ps_qkv0[cb][:],
                                    w_sb[:, h, bass.ts(cb, 128)],
                                    hst_tiles[h // 8][:, h % 8, :],
                                    start=(h == 0), stop=(h == HCH - 1),
                                )
                            if g == 3:
                                qkv_post(cb, ps_qkv0[cb])
                                if cb == 2:
                                    v_transposes()
                            if g == 0 and cb == 0:
                                nc.scalar.dma_start(cos_sb[:], cos_d[:])
                                nc.scalar.dma_start(sin_sb[:], sin_d[:])
                                nc.scalar.dma_start(mask_sb[:], mask_d[:])
                else:
                    for cb in range(CB):
                        wt = w_sb[:, :, bass.ts(cb, 128)]
                        ps = psp.tile([128, TJ], F32, tag="ps", name="ps_qkv")
                        for h in range(HCH):
                            nc.tensor.matmul(
                                ps[:], wt[:, h, :],
                                hst_tiles[4 * j + h // 8][:, h % 8, :],
                                start=(h == 0), stop=(h == HCH - 1),
                            )
                        qkv_post(cb, ps)
                        if cb == 2:
                            v_transposes()
                # ---- attention for this t tile ----
                nblk = 4 * j + 4

                def normalize(h, ps_o, ps_den):
                    # 1/den = exp(-ln(den)) on ACT (vector.reciprocal is slow)
                    lden = rcpp.tile([128, TJ], F32, tag="lden", name="lden")
                    nc.scalar.activation(
                        lden[:], ps_den[:], mybir.ActivationFunctionType.Ln
                    )
                    rc = rcpp.tile([128, TJ], F32, tag="rc", name="rc")
                    nc.scalar.activation(
                        rc[:], lden[:], mybir.ActivationFunctionType.Exp,
                        scale=-1.0,
                    )
                    nc.vector.tensor_tensor(
                        ot_strip[(j, h)][:], ps_o[:], rc[:], mybir.AluOpType.mult
                    )

                if j == 0:
                    issue_hst_upto(8)  # rest of j1's chunks ahead of wo
                    for hh in range(NQH):
                        nc.scalar.dma_start(wo_sb[:, hh, :], wo_r[:, hh, :])

                for h in range(NQH):
                    qT = q_strip[h]
                    ot_strip[(j, h)] = qsp.tile(
                        [128, TJ], BF16, tag=f"ot{h}", name=f"ot{h}_{j}"
                    )

                    def sc_exp(i):
                        # trapezoid: diagonal s-block r only feeds t >= 128r
                        diag = i >= 4 * j
                        r = i - 4 * j
                        w = TJ - 128 * r if diag else TJ
                        ps_s = psp.tile([128, TJ], F32, tag="ps", name="ps_s")
                        nc.tensor.matmul(
                            ps_s[:, 0:w],
                            kT_sb[i // 4][:, bass.ts(i % 4, 128)],
                            qT[:, bass.ds(TJ - w, w)],
                            start=True, stop=True,
                        )
                        p_ij = ppp.tile([128, TJ], BF16, tag="p")
                        nc.scalar.activation(
                            p_ij[:, 0:w], ps_s[:, 0:w],
                            mybir.ActivationFunctionType.Exp, scale=SCALE,
                        )
                        if diag:
                            # triangular mask on the first 128 cols (s==t block)
                            nc.vector.tensor_tensor(
                                p_ij[:, 0:128], p_ij[:, 0:128], mask_sb[:],
                                mybir.AluOpType.mult,
                            )
                        return p_ij

                    def pv_den(i, p_ij, ps_o, ps_den):
                        w = TJ - 128 * (i - 4 * j) if i >= 4 * j else TJ
                        osl = bass.ds(TJ - w, w)
                        nc.tensor.matmul(
                            ps_o[:, osl], v_sb[i][:], p_ij[:, 0:w],
                            start=(i == 0), stop=(i == nblk - 1),
                        )
                        nc.tensor.matmul(
                            ps_den[:, osl], ones_sb[:], p_ij[:, 0:w],
                            start=(i == 0), stop=(i == nblk - 1),
                        )

                    # prologue: two scores+exp ahead so the exp pipeline is
                    # primed when PV work arrives after the o_proj filler
                    pre = [sc_exp(i) for i in range(min(2, nblk))]
                    # o_proj t-block of the previous strip: PE filler while
                    # this head's exps stream on ACT
                    if j > 0:
                        for n in range(HID // TJ):
                            oproj_tile(4 * (j - 1) + h, n)
                    ps_o = psp.tile([128, TJ], F32, tag="ps", name="ps_o")
                    ps_den = psp.tile([128, TJ], F32, tag="ps", name="ps_den")
                    for i in range(nblk):
                        p_ij = pre[i] if i < len(pre) else sc_exp(i)
                        pv_den(i, p_ij, ps_o, ps_den)
                    normalize(h, ps_o, ps_den)

            for t in range(4 * (NJ - 1), 4 * NJ):
                for n in range(HID // TJ):
                    if t == 4 * NJ - 1 and n == HID // TJ - 1:
                        # split the very last tile: copy + DMA in quarters so
                        # the final drain waits on a 128-col transfer
                        tj, tr = t // 4, bass.ts(t % 4, 128)
                        ps = psp.tile([128, TJ], F32, tag="ps", name="ps_op")
                        for c in range(NQH):
                            nc.tensor.matmul(
                                ps[:], ot_strip[(tj, c)][:, tr],
                                wo_sb[:, c, bass.ts(n, TJ)],
                                start=(c == 0), stop=(c == NQH - 1),
                            )
                        ob = outp.tile([128, TJ], BF16, tag="out")
                        for q in range(4):
                            qs_ = bass.ds(q * 128, 128)
                            nc.vector.tensor_copy(ob[:, qs_], ps[:, qs_])
                            nc.sync.dma_start(
                                out_d[bass.ts(t, 128),
                                      bass.ds(n * TJ + q * 128, 128)],
                                ob[:, qs_],
                            )
                    else:
                        oproj_tile(t, n)

    _orig_to_json = nc.to_json_bytes

    def _patched_to_json():
        return _fix_bir_waits(_orig_to_json())

    nc.to_json_bytes = _patched_to_json
    return nc


_NC_CACHE = None


def _get_nc():
    global _NC_CACHE
    if _NC_CACHE is None:
        _NC_CACHE = build_nc()
    return _NC_CACHE


def _host_prep(positions, hidden_states, w_qkv, w_o):
    H, HKV = 32, 8
    pos = np.asarray(positions).astype(np.float32)
    inv_freq = (
        1.0 / (500000.0 ** (np.arange(0, D, 2, dtype=np.float32) / D))
    )
    freqs = pos[:, None] * inv_freq[None, :]                  # [T, 64]
    cos = np.cos(freqs).T                                     # [64, T]
    sin = np.sin(freqs).T
    cos2 = np.concatenate([cos, cos], 0).astype(bf16)         # [128, T]
    sin2 = np.concatenate([-sin, sin], 0).astype(bf16)

    # triangular 0/1 mask for the s==t diagonal 128-block: keep s <= t
    p = np.arange(128)[:, None]
    f = np.arange(128)[None, :]
    masktri = (p <= f).astype(np.float32).astype(bf16)        # [128, 128]

    hsT = np.ascontiguousarray(np.asarray(hidden_states).T).astype(bf16)
    w_qkv = np.asarray(w_qkv)
    w_o = np.asarray(w_o)

    in_maps = []
    for core in range(8):
        qc = slice(core * 4 * D, (core + 1) * 4 * D)
        kc = slice(H * D + core * D, H * D + (core + 1) * D)
        vc = slice((H + HKV) * D + core * D, (H + HKV) * D + (core + 1) * D)
        # column-block order: v, k, q0..q3
        wshard = np.concatenate(
            [w_qkv[:, vc], w_qkv[:, kc], w_qkv[:, qc]], axis=1
        ).astype(bf16)
        woshard = np.ascontiguousarray(
            w_o[core * 512:(core + 1) * 512, :]
        ).astype(bf16)
        in_maps.append(
            {
                "hsT": hsT,
                "wqkv": wshard,
                "wo": woshard,
                "cos2": cos2,
                "sin2": sin2,
                "masktri": masktri,
            }
        )
    return in_maps


def kernel(positions, hidden_states, w_qkv, w_o, _trace=False):
    nc = _get_nc()
    in_maps = _host_prep(positions, hidden_states, w_qkv, w_o)
    res = run_bass_kernel_spmd(nc, in_maps, list(range(8)), trace=_trace)
    out = np.zeros((T, HID), np.float32)
    for c in range(8):
        out += res.results[c]["out"].astype(np.float32)
    if _trace:
        kernel._last_result = res
    return out


# revision 28
# speedup vs baseline: 1.0394x; 1.0394x over previous
"""Llama3 attention layer (T=2048, 32 q heads / 8 kv heads, D=128, hidden 4096)
on 8 Trainium2 NeuronCores, tensor-parallel over heads.

Per-core shard: 4 q heads + 1 kv head (w_qkv columns), 512 w_o rows.
Each core computes a full [T, 4096] o_proj partial in bf16; the host sums
the 8 partials in fp32 (the all-reduce of the row-parallel w_o).

Device algorithm (per core), all matmuls in bf16 with fp32 PSUM accumulation:
  1. qkv^T = w_shard^T @ hs^T          -> [768, T]  (c on partitions)
     column-block order v, k, q0..q3; w_qkv resident in SBUF (loaded once);
     hs^T streamed through a 6-deep chunk ring with one-tile lookahead
  2. RoPE on q^T/k^T rows via duplicated cos/sin tables (DVE)
  3. V = transpose(v^T) via PE transposes
  4. S^T[s,t] = k^T.T @ q^T (per head), exp on ACT; diagonal s-blocks use
     trapezoid moving slices (width 512-128r) + one [128,128] triangular mask
  5. out^T[d,t] += V[s].T @ P^T[s,t]; denom via ones-matmul;
     1/den = exp(-ln(den)) on ACT; normalize mult on DVE
  6. out[t,:] = (O^T).T @ w_o_shard    -> bf16 partial, DMA to DRAM
"""
import math

import numpy as np
import ml_dtypes

import bass_rust
import concourse.bass as bass
import concourse.mybir as mybir
import concourse.tile as tile
from concourse.bass_utils import run_bass_kernel_spmd
from concourse.masks import make_identity
from concourse.vector_clock import ScopedClock

BF16 = mybir.dt.bfloat16
F32 = mybir.dt.float32
bf16 = ml_dtypes.bfloat16

T = 2048
HID = 4096
D = 128
NQH = 4          # q heads per core
CB = 6           # qkv col blocks of 128 (v, k, 4 q heads)
HCH = HID // 128  # 32 hidden chunks
TJ = 512          # t tile width
NJ = T // TJ      # 4 t tiles
SB = T // 128     # 16 s blocks
SCALE = 1.0 / math.sqrt(D)

_MAX_CTRL_WAITS = 1


def _install_drain_fix():
    """walrus in this image allows only 1 sem wait on CTRL (nop/drain)
    instructions; spread the Tile tail-drain's global-clock waits across
    preceding sync-engine NOPs."""
    if getattr(tile.TileContext, "_drain_fix_installed", False):
        return

    def _patched(self, tick_clock, wait_clock):
        nc = self.nc
        nops = [nc.sync.nop(nofuse=True, hint=f"drainw{i}") for i in range(32)]
        drain_inst = nc.sync.drain()
        wait_clock.add_sem_waits(
            drain_inst.ins, ScopedClock({None: tick_clock.global_clock})
        )
        si = drain_inst.ins.sync_info
        waits = list(si.on_wait) if si and si.on_wait else []
        if len(waits) > _MAX_CTRL_WAITS:
            chunks = [
                waits[i:i + _MAX_CTRL_WAITS]
                for i in range(0, len(waits), _MAX_CTRL_WAITS)
            ]
            drain_inst.ins.sync_info = bass_rust.SyncInfo(
                on_wait=chunks[-1], on_update=list(si.on_update or [])
            )
            for nop, chunk in zip(nops, chunks[:-1]):
                nop.ins.sync_info = bass_rust.SyncInfo(on_wait=chunk, on_update=[])
        nc.all_engine_barrier()
        assert self.sems is not None
        popped = nc._tile_sem_poison_stack.pop()
        assert popped is self._sem_poison
        nc.clear_and_free_semaphores(list(self.sems.allocated().values()))
        nc.all_engine_barrier()

    tile.TileContext._drain_and_barrier = _patched
    tile.TileContext._drain_fix_installed = True


def _fix_bir_waits(bir_json: bytes, max_waits: int = 1) -> bytes:
    """walrus in this image accepts very few sem waits per instruction.
    Split any instruction carrying more than `max_waits` waits by inserting
    same-engine NoOps ahead of it that carry the excess waits."""
    import json

    bir = json.loads(bir_json)
    n_split = 0
    for fn in bir["functions"]:
        for blk in fn["blocks"]:
            out = []
            for inst in blk["instructions"]:
                si = inst.get("sync_info")
                waits = (si or {}).get("on_wait") or []
                if len(waits) > max_waits:
                    chunks = [
                        waits[i:i + max_waits]
                        for i in range(0, len(waits), max_waits)
                    ]
                    for k, ch in enumerate(chunks[:-1]):
                        out.append(
                            {
                                "debug": inst.get("debug", 0),
                                "engine": inst["engine"],
                                "ins": [],
                                "name": f"{inst['name']}-w{k}",
                                "opcode": "NoOp",
                                "outs": [],
                                "sync_info": {"on_update": [], "on_wait": ch},
                            }
                        )
                        n_split += 1
                    si["on_wait"] = chunks[-1]
                out.append(inst)
            blk["instructions"] = out
    return json.dumps(bir).encode()


def build_nc() -> bass.Bass:
    _install_drain_fix()
    nc = bass.Bass()

    hsT_d = nc.dram_tensor("hsT", [HID, T], BF16, kind="ExternalInput")
    wqkv_d = nc.dram_tensor("wqkv", [HID, CB * 128], BF16, kind="ExternalInput")
    wo_d = nc.dram_tensor("wo", [NQH * 128, HID], BF16, kind="ExternalInput")
    cos_d = nc.dram_tensor("cos2", [128, T], BF16, kind="ExternalInput")
    sin_d = nc.dram_tensor("sin2", [128, T], BF16, kind="ExternalInput")
    mask_d = nc.dram_tensor("masktri", [128, 128], BF16, kind="ExternalInput")
    out_d = nc.dram_tensor("out", [T, HID], BF16, kind="ExternalOutput")

    hsT_r = hsT_d.rearrange("(o p) t -> p o t", p=128)     # [128, 32, T]
    wqkv_r = wqkv_d.rearrange("(o p) c -> p o c", p=128)   # [128, 32, 768]
    wo_r = wo_d.rearrange("(o p) n -> p o n", p=128)       # [128, 4, HID]

    with tile.TileContext(nc) as tc:
        with (
            tc.tile_pool(name="const", bufs=1) as constp,
            tc.tile_pool(name="acts", bufs=1) as actp,
            tc.tile_pool(name="ps", bufs=8, space="PSUM") as psp,
            tc.tile_pool(name="hst", bufs=6) as hstp,
            tc.tile_pool(name="qs", bufs=2) as qsp,
            tc.tile_pool(name="qkt", bufs=2) as qktp,
            tc.tile_pool(name="rtmp", bufs=2) as rtp,
            tc.tile_pool(name="pp", bufs=10) as ppp,
            tc.tile_pool(name="rcp", bufs=1) as rcpp,
            tc.tile_pool(name="outp", bufs=4) as outp,
        ):
            cos_sb = constp.tile([128, T], BF16, tag="cos")
            sin_sb = constp.tile([128, T], BF16, tag="sin")
            mask_sb = constp.tile([128, 128], BF16, tag="masktri")
            ones_sb = constp.tile([128, 128], BF16, tag="ones")
            ident_sb = constp.tile([128, 128], BF16, tag="ident")
            warm_sb = constp.tile([128, TJ], BF16, tag="warm")

            # PE warmup: dummy matmuls fill the initial DMA wait so the HAM
            # clock-gate opens before real work arrives (~3.4us busy needed).
            nc.vector.memset(warm_sb[:], 0.0)
            for wi in range(14):
                pw = psp.tile([128, TJ], F32, tag="ps", name="ps_warm")
                nc.tensor.matmul(
                    pw[:], warm_sb[:, 0:128], warm_sb[:], start=True, stop=True
                )
            nc.vector.memset(ones_sb[:], 1.0)
            make_identity(nc, ident_sb[:])

            # persistent activations (per-strip tiles avoid false cross-strip
            # dependencies in the tile tracker)
            w_sb = actp.tile([128, HCH, 768], BF16, tag="w")
            wo_sb = actp.tile([128, NQH, HID], BF16, tag="wo")
            kT_sb = [
                actp.tile([128, TJ], BF16, tag=f"kT{jj}", name=f"kT{jj}")
                for jj in range(NJ)
            ]
            v_sb = [actp.tile([128, 128], BF16, tag=f"v{i}", name=f"v{i}") for i in range(SB)]

            # hs^T chunk ring: 16 chunks of [128, 8, 512], one-j lookahead
            hst_tiles = [None] * (4 * NJ)
            hst_issued = [0]

            def issue_hst_upto(limit):
                while hst_issued[0] < min(limit, 4 * NJ):
                    g = hst_issued[0]
                    tile_g = hstp.tile(
                        [128, 8, TJ], BF16, tag="hst", name=f"hst{g}"
                    )
                    if g == 0:
                        # split the very first chunk so the first QKV matmuls
                        # can start half a transfer earlier
                        nc.sync.dma_start(
                            tile_g[:, 0:4, :], hsT_r[:, 0:4, 0:TJ]
                        )
                        nc.sync.dma_start(
                            tile_g[:, 4:8, :], hsT_r[:, 4:8, 0:TJ]
                        )
                    else:
                        nc.sync.dma_start(
                            tile_g[:],
                            hsT_r[:, (g % 4) * 8:(g % 4 + 1) * 8, bass.ts(g // 4, TJ)],
                        )
                    hst_tiles[g] = tile_g
                    hst_issued[0] += 1

            ot_strip = {}  # (j, h) -> strip tile, live for one j

            def oproj_tile(t, n):
                tj, tr = t // 4, bass.ts(t % 4, 128)
                ps = psp.tile([128, TJ], F32, tag="ps", name="ps_op")
                for c in range(NQH):
                    nc.tensor.matmul(
                        ps[:], ot_strip[(tj, c)][:, tr], wo_sb[:, c, bass.ts(n, TJ)],
                        start=(c == 0), stop=(c == NQH - 1),
                    )
                ob = outp.tile([128, TJ], BF16, tag="out")
                nc.vector.tensor_copy(ob[:], ps[:])
                nc.sync.dma_start(out_d[bass.ts(t, 128), bass.ts(n, TJ)], ob[:])

            for j in range(NJ):
                js = bass.ts(j, TJ)
                # ---- QKV^T for this t tile ----
                if j == 0:
                    # interleave w octets with the first hst chunks so the
                    # first accumulation chain starts as soon as chunk 0 lands
                    nc.sync.dma_start(w_sb[:, 0:4, :], wqkv_r[:, 0:4, :])
                    issue_hst_upto(1)
                    nc.sync.dma_start(w_sb[:, 4:8, :], wqkv_r[:, 4:8, :])
                    for g in range(1, 4):
                        issue_hst_upto(g + 1)
                        nc.sync.dma_start(
                            w_sb[:, g * 8:(g + 1) * 8, :],
                            wqkv_r[:, g * 8:(g + 1) * 8, :],
                        )
                    issue_hst_upto(8)
                else:
                    issue_hst_upto(4 * (j + 1) + 2)

                # per-strip q tiles (roped q heads for this j)
                q_strip = [
                    qsp.tile([128, TJ], BF16, tag=f"q{h}", name=f"q{h}_{j}")
                    for h in range(NQH)
                ]
                vT_j = qsp.tile([128, TJ], BF16, tag="vT", name=f"vT_{j}")

                def v_transposes():
                    # PE-transpose the V strip to [s, d] blocks; deferred two
                    # chains past cb0 so the PE isn't stalled on the vT copy
                    for r in range(4):
                        pv = psp.tile([128, 128], BF16, tag="ps", name="ps_vt")
                        nc.tensor.transpose(
                            pv[:], vT_j[:, bass.ts(r, 128)], ident_sb[:]
                        )
                        nc.vector.tensor_copy(v_sb[4 * j + r][:], pv[:])

                def qkv_post(cb, ps):
                    if cb == 0:
                        # V block: copy to SBUF (transposes deferred)
                        nc.vector.tensor_copy(vT_j[:], ps[:])
                    else:
                        # k (cb=1) -> kT_sb strip; q head h (cb=2+h) -> q_strip
                        qk_t = qktp.tile([128, TJ], BF16, tag="qkt")
                        nc.vector.tensor_copy(qk_t[:], ps[:])
                        # rope: q' = q*cos2 + swap(q)*sin2 (sin2 top half negated)
                        swp = rtp.tile([128, TJ], BF16, tag="swp")
                        nc.vector.tensor_copy(swp[0:64, :], qk_t[64:128, :])
                        nc.vector.tensor_copy(swp[64:128, :], qk_t[0:64, :])
                        ta = rtp.tile([128, TJ], BF16, tag="ta")
                        nc.vector.tensor_tensor(
                            ta[:], qk_t[:], cos_sb[:, js], mybir.AluOpType.mult
                        )
                        tb = rtp.tile([128, TJ], BF16, tag="tb")
                        nc.vector.tensor_tensor(
                            tb[:], swp[:], sin_sb[:, js], mybir.AluOpType.mult
                        )
                        if cb == 1:
                            nc.vector.tensor_tensor(
                                kT_sb[j][:], ta[:], tb[:], mybir.AluOpType.add
                            )
                        else:
                            nc.vector.tensor_tensor(
                                q_strip[cb - 2][:], ta[:], tb[:],
                                mybir.AluOpType.add,
                            )

                if j == 0:
                    # DMA-bound pipeline head: octet-group outer, cb inner —
                    # 6 parallel PSUM chains give 10us of PE work per arriving
                    # (w octet, hst chunk) pair instead of 1.7us
                    ps_qkv0 = [
                        psp.tile([128, TJ], F32, tag="ps", name=f"ps_qkv0_{cb}")
                        for cb in range(CB)
                    ]
                    for g in range(4):
                        for cb in range(CB):
                            for h in range(8 * g, 8 * g + 8):
                                nc.tensor.matmul(
                                    ps_qkv0[cb][:],
                                    w_sb[:, h, bass.ts(cb, 128)],
                                    hst_tiles[h // 8][:, h % 8, :],
                                    start=(h == 0), stop=(h == HCH - 1),
                                )
                            if g == 3:
                                qkv_post(cb, ps_qkv0[cb])
                                if cb == 2:
                                    v_transposes()
                            if g == 0 and cb == 0:
                                nc.sync.dma_start(cos_sb[:], cos_d[:])
                                nc.sync.dma_start(sin_sb[:], sin_d[:])
                                nc.sync.dma_start(mask_sb[:], mask_d[:])
                else:
                    for cb in range(CB):
                        wt = w_sb[:, :, bass.ts(cb, 128)]
                        ps = psp.tile([128, TJ], F32, tag="ps", name="ps_qkv")
                        for h in range(HCH):
                            nc.tensor.matmul(
                                ps[:], wt[:, h, :],
                                hst_tiles[4 * j + h // 8][:, h % 8, :],
                                start=(h == 0), stop=(h == HCH - 1),
                            )
                        qkv_post(cb, ps)
                        if cb == 2:
                            v_transposes()
                # ---- attention for this t tile ----
                nblk = 4 * j + 4

                def normalize(h, ps_o, ps_den):
                    # 1/den = exp(-ln(den)) on ACT (vector.reciprocal is slow)
                    lden = rcpp.tile([128, TJ], F32, tag="lden", name="lden")
                    nc.scalar.activation(
                        lden[:], ps_den[:], mybir.ActivationFunctionType.Ln
                    )
                    rc = rcpp.tile([128, TJ], F32, tag="rc", name="rc")
                    nc.scalar.activation(
                        rc[:], lden[:], mybir.ActivationFunctionType.Exp,
                        scale=-1.0,
                    )
                    nc.vector.tensor_tensor(
                        ot_strip[(j, h)][:], ps_o[:], rc[:], mybir.AluOpType.mult
                    )

                if j == 0:
                    issue_hst_upto(8)  # rest of j1's chunks ahead of wo
                    for hh in range(NQH):
                        nc.sync.dma_start(wo_sb[:, hh, :], wo_r[:, hh, :])

                for h in range(NQH):
                    qT = q_strip[h]
                    ot_strip[(j, h)] = qsp.tile(
                        [128, TJ], BF16, tag=f"ot{h}", name=f"ot{h}_{j}"
                    )

                    def sc_exp(i):
                        # trapezoid: diagonal s-block r only feeds t >= 128r
                        diag = i >= 4 * j
                        r = i - 4 * j
                        w = TJ - 128 * r if diag else TJ
                        ps_s = psp.tile([128, TJ], F32, tag="ps", name="ps_s")
                        nc.tensor.matmul(
                            ps_s[:, 0:w],
                            kT_sb[i // 4][:, bass.ts(i % 4, 128)],
                            qT[:, bass.ds(TJ - w, w)],
                            start=True, stop=True,
                        )
                        p_ij = ppp.tile([128, TJ], BF16, tag="p")
                        nc.scalar.activation(
                            p_ij[:, 0:w], ps_s[:, 0:w],
                            mybir.ActivationFunctionType.Exp, scale=SCALE,
                        )
                        if diag:
                            # triangular mask on the first 128 cols (s==t block)
                            nc.vector.tensor_tensor(
                                p_ij[:, 0:128], p_ij[:, 0:128], mask_sb[:],
                                mybir.AluOpType.mult,
                            )
                        return p_ij

                    def pv_den(i, p_ij, ps_o, ps_den):
                        w = TJ - 128 * (i - 4 * j) if i >= 4 * j else TJ
                        osl = bass.ds(TJ - w, w)
                        nc.tensor.matmul(
                            ps_o[:, osl], v_sb[i][:], p_ij[:, 0:w],
                            start=(i == 0), stop=(i == nblk - 1),
                        )
                        nc.tensor.matmul(
                            ps_den[:, osl], ones_sb[:], p_ij[:, 0:w],
                            start=(i == 0), stop=(i == nblk - 1),
                        )

                    # prologue: two scores+exp ahead so the exp pipeline is
                    # primed when PV work arrives after the o_proj filler
                    pre = [sc_exp(i) for i in range(min(2, nblk))]
                    # o_proj t-block of the previous strip: PE filler while
                    # this head's exps stream on ACT
                    if j > 0:
                        for n in range(HID // TJ):
                            oproj_tile(4 * (j - 1) + h, n)
                    ps_o = psp.tile([128, TJ], F32, tag="ps", name="ps_o")
                    ps_den = psp.tile([128, TJ], F32, tag="ps", name="ps_den")
                    for i in range(nblk):
                        p_ij = pre[i] if i < len(pre) else sc_exp(i)
                        pv_den(i, p_ij, ps_o, ps_den)
                    normalize(h, ps_o, ps_den)

            for t in range(4 * (NJ - 1), 4 * NJ):
                for n in range(HID // TJ):
                    if t == 4 * NJ - 1 and n == HID // TJ - 1:
                        # split the very last tile: copy + DMA in quarters so
                        # the final drain waits on a 128-col transfer
                        tj, tr = t // 4, bass.ts(t % 4, 128)
                        ps = psp.tile([128, TJ], F32, tag="ps", name="ps_op")
                        for c in range(NQH):
                            nc.tensor.matmul(
                                ps[:], ot_strip[(tj, c)][:, tr],
                                wo_sb[:, c, bass.ts(n, TJ)],
                                start=(c == 0), stop=(c == NQH - 1),
                            )
                        ob = outp.tile([128, TJ], BF16, tag="out")
                        for q in range(4):
                            qs_ = bass.ds(q * 128, 128)
                            nc.vector.tensor_copy(ob[:, qs_], ps[:, qs_])
                            nc.sync.dma_start(
                                out_d[bass.ts(t, 128),
                                      bass.ds(n * TJ + q * 128, 128)],
                                ob[:, qs_],
                            )
                    else:
                        oproj_tile(t, n)

    _orig_to_json = nc.to_json_bytes

    def _patched_to_json():
        return _fix_bir_waits(_orig_to_json())

    nc.to_json_bytes = _patched_to_json
    return nc


_NC_CACHE = None


def _get_nc():
    global _NC_CACHE
    if _NC_CACHE is None:
        _NC_CACHE = build_nc()
    return _NC_CACHE


def _host_prep(positions, hidden_states, w_qkv, w_o):
    H, HKV = 32, 8
    pos = np.asarray(positions).astype(np.float32)
    inv_freq = (
        1.0 / (500000.0 ** (np.arange(0, D, 2, dtype=np.float32) / D))
    )
    freqs = pos[:, None] * inv_freq[None, :]                  # [T, 64]
    cos = np.cos(freqs).T                                     # [64, T]
    sin = np.sin(freqs).T
    cos2 = np.concatenate([cos, cos], 0).astype(bf16)         # [128, T]
    sin2 = np.concatenate([-sin, sin], 0).astype(bf16)

    # triangular 0/1 mask for the s==t diagonal 128-block: keep s <= t
    p = np.arange(128)[:, None]
    f = np.arange(128)[None, :]
    masktri = (p <= f).astype(np.float32).astype(bf16)        # [128, 128]

    hsT = np.ascontiguousarray(np.asarray(hidden_states).T).astype(bf16)
    w_qkv = np.asarray(w_qkv)
    w_o = np.asarray(w_o)

    in_maps = []
    for core in range(8):
        qc = slice(core * 4 * D, (core + 1) * 4 * D)
        kc = slice(H * D + core * D, H * D + (core + 1) * D)
        vc = slice((H + HKV) * D + core * D, (H + HKV) * D + (core + 1) * D)
        # column-block order: v, k, q0..q3
        wshard = np.concatenate(
            [w_qkv[:, vc], w_qkv[:, kc], w_qkv[:, qc]], axis=1
        ).astype(bf16)
        woshard = np.ascontiguousarray(
            w_o[core * 512:(core + 1) * 512, :]
        ).astype(bf16)
        in_maps.append(
            {
                "hsT": hsT,
                "wqkv": wshard,
                "wo": woshard,
                "cos2": cos2,
                "sin2": sin2,
                "masktri": masktri,
            }
        )
    return in_maps


def kernel(positions, hidden_states, w_qkv, w_o, _trace=False):
    nc = _get_nc()
    in_maps = _host_prep(positions, hidden_states, w_qkv, w_o)
    res = run_bass_kernel_spmd(nc, in_maps, list(range(8)), trace=_trace)
    out = np.zeros((T, HID), np.float32)
    for c in range(8):
        out += res.results[c]["out"].astype(np.float32)
    if _trace:
        kernel._last_result = res
    return out
